# revision 15
# baseline (speedup 1.0000x reference)
"""AdaptiveRSNNEmbedding Trainium2 kernel v4 (8 NeuronCores, batch-parallel).

Reference semantics (per batch element, T time-reversed steps):
    g, c   = split(conv3x3(spike_prev, w_gate) + conv3x3(ev_t, w_in) + biases)
    gate   = sigmoid(g);  v = gate*vmem + c
    spike  = (v > 0.5);   vmem' = v - 0.5*spike
    agg[seg] accumulates vavg at spikes (seg<4), seg += spike, plus a final
    flush of the unclosed segment.

Identity: agg[s] += v_t * (seg_t == s) each step (seg_t = #spikes before t,
uncapped) replaces vavg/scatter/final-flush exactly.

v4 design vs v2 (2762760ns):
  * conv unchanged (fp16 hi+lo passes, ky-stacked gathers): chaos analysis
    shows the dynamics amplify any conv error >~1e-6 into >2e-2 output
    error, so fp8/f32r cheaper matmuls are not usable. PE ~64us/step is the
    floor; everything else must hide under it.
  * stationary columns per tile: hbA = [g|c], hbB = [c|g], so each psum
    pair tile has both cur blocks contiguous (partitions 32:96): extraction
    is one [64,F] Identity + two [32,F] Sigmoids per pair on Act (32us/step
    vs 42.7 baseline), all hidden under PE. (DMA cannot read PSUM.)
  * biases ride a constant-1.0 K-row (row 108) with (bg+bi) hi/lo in the
    kx=1 stationary blocks; no bias ops at extraction.
  * all elementwise work uses scalar_tensor_tensor/tensor_scalar forms:
    DVE 2x_2p mode (any dtype, all-SBUF) or 4x_2p (all 2-byte): v chain
    f32 at 0.52ns/elem, agg mask+add fp16 at 0.26ns/elem. vmem/seg updates
    on Pool as STT (eff 0.6) instead of tensor_tensor Add (eff 0.42).
  * agg planes fp16 (|agg|<~1500 fits; 4x better accumulation than bf16).
  * output DMAs fused across hb (4-dim APs): 8 DMAs per b on the Act queue.
"""
import sys
import time
import numpy as np

if '/opt/trn_rl_repo' not in sys.path:
    sys.path.insert(0, '/opt/trn_rl_repo')

import concourse.bass as bass
import concourse.mybir as mybir
from concourse.tile import TileContext

F32 = mybir.dt.float32
F16 = mybir.dt.float16
BF16 = mybir.dt.bfloat16
U8 = mybir.dt.uint8
AF = mybir.ActivationFunctionType
OP = mybir.AluOpType

# problem constants
B, T, CIN, COUT, H, W = 16, 16, 2, 32, 160, 160
TSEG = 4
NCORES = 8
BL = B // NCORES
THRESH = 0.5

# kept for test.py compatibility
CONV_DT = F32
CONV_MODE = "fp16hl"


def mkap(base_ap, offset, dims):
    return type(base_ap)(base_ap.tensor, offset, [list(d) for d in dims])


def build_nc(BL=BL, T=T, H=H, W=W, conv_dt=None, mode=None):
    HB = 4
    BR = H // HB                # rows per block (40)
    HR = BR // 2                # rows per half-step (20)
    NPIX = BR * W               # per-partition state pixels (6400)
    WP = W + 2                  # padded row width (162)
    HP = H + 2                  # padded event-plane rows
    SR = max(1, 512 // W)       # matmul slice rows (psum bank limit)
    SSF = HR * WP               # ss cols per hb (3240)
    KROWS = 109                 # 96 spike + 12 event + 1 bias

    chunks = []                 # (c0, cr) within a half
    r = 0
    while r < HR:
        cr = min(2 * SR, HR - r)
        chunks.append((r, cr))
        r += cr

    nc = bass.Bass()
    nop_sem = nc.semaphore("nopsem").__enter__()
    ev_d = nc.declare_dram_parameter("ev", [BL * T, 12, HP * WP], F16,
                                     isOutput=False)
    ww_d = nc.declare_dram_parameter("ww", [128, 384], F16, isOutput=False)
    out_d = nc.declare_dram_parameter("out", [TSEG, BL, COUT, H, W], F16,
                                      isOutput=True)

    with TileContext(nc) as tc:
        with tc.tile_pool(name="const", bufs=1) as cpool, \
             tc.tile_pool(name="state", bufs=1) as spool, \
             tc.tile_pool(name="gc", bufs=2) as gcpool, \
             tc.tile_pool(name="wv", bufs=4) as vpool, \
             tc.tile_pool(name="wb", bufs=2) as vbpool, \
             tc.tile_pool(name="wmv", bufs=1) as mvpool, \
             tc.tile_pool(name="psum", bufs=2, space="PSUM") as ppool:

            ww_t = cpool.tile([128, 384], F16, tag="ww")
            nc.sync.dma_start(out=ww_t[:], in_=ww_d[:])

            vmem = spool.tile([128, NPIX], F32, tag="vmem")
            seg = spool.tile([128, NPIX], F16, tag="seg")
            aggs = [spool.tile([128, NPIX], F16, tag=f"agg{s}",
                               name=f"agg{s}") for s in range(TSEG)]
            spk = spool.tile([128, (BR + 2) * WP], F16, tag="spk")
            sss = [spool.tile([KROWS, 4 * SSF], F16, tag=f"ss{hf}",
                              name=f"ss{hf}") for hf in range(2)]

            nc.nop_sem_num = nop_sem.num
            spk_v = spk[:].rearrange("p (r x) -> p r x", x=WP)

            # pad cols/rows of spk must read 0; interior rows are fully
            # rewritten at t==0 of every b, so one memset suffices globally
            nc.gpsimd.memset(spk[:].bitcast(F32), 0.0)
            # bias row: constant 1.0 across all hb planes (never rewritten)
            for hf in range(2):
                nc.gpsimd.memset(sss[hf][96:109, :], 1.0)

            for b in range(BL):

                def issue_gather(hf, ky, rlo, rhi):
                    # ss rows [rlo, rhi) of ky plane <- spk rows h0+ky+r
                    ss = sss[hf]
                    h0 = hf * HR
                    for hb in range(4):
                        nc.sync.dma_start(
                            out=ss[32 * ky:32 * (ky + 1),
                                   hb * SSF + rlo * WP:hb * SSF + rhi * WP],
                            in_=spk[32 * hb:32 * (hb + 1),
                                    (h0 + ky + rlo) * WP:
                                    (h0 + ky + rhi) * WP])

                def issue_top_edge():
                    # ssA ky0 row 0 <- block hb-1 row 39 (spk row 40), from
                    # the source partitions directly (no guard-row writes)
                    ss = sss[0]
                    for hb in range(4):
                        src = (spk[0:32, 0:WP] if hb == 0 else
                               spk[32 * (hb - 1):32 * hb,
                                   BR * WP:(BR + 1) * WP])
                        nc.sync.dma_start(
                            out=ss[0:32, hb * SSF:hb * SSF + WP], in_=src)

                def issue_bot_edge():
                    # ssB ky2 row HR-1 <- block hb+1 row 0 (spk row 1)
                    ss = sss[1]
                    for hb in range(4):
                        src = (spk[96:128, (BR + 1) * WP:(BR + 2) * WP]
                               if hb == 3 else
                               spk[32 * (hb + 1):32 * (hb + 2), WP:2 * WP])
                        nc.sync.dma_start(
                            out=ss[64:96,
                                   hb * SSF + (HR - 1) * WP:(hb + 1) * SSF],
                            in_=src)

                def issue_ev(hf, bt):
                    ss = sss[hf]
                    h0 = hf * HR
                    in_ev = mkap(
                        ev_d[:], bt * 12 * HP * WP + h0 * WP,
                        [(HP * WP, 12), (BR * WP, 4), (1, SSF)])
                    out_ev = ss[96:108, :].rearrange(
                        "p (hb rx) -> p hb rx", hb=4, rx=SSF)
                    nc.sync.dma_start(out=out_ev, in_=in_ev)

                def issue_out(h0w, h1w):
                    for s in range(TSEG):
                        for hb in range(4):
                            nc.gpsimd.dma_start(
                                out=out_d[s, b, :,
                                          hb * BR + h0w:hb * BR + h1w, :],
                                in_=aggs[s][32 * hb:32 * (hb + 1),
                                            h0w * W:h1w * W])

                def do_phase2(t, sl, F, v_ap, spk_sl):
                    # vmem' = v - spike  (Pool STT: 0.83ns/elem)
                    nc.gpsimd.scalar_tensor_tensor(
                        vmem[:, sl], spk_sl, -1.0, v_ap,
                        OP.mult, OP.add)
                    # vb = fp16 copy of v (DVE tensor_scalar: 2x_2p)
                    vb_t = vbpool.tile([128, F], F16, tag="vb")
                    nc.vector.tensor_scalar(vb_t[:, :F], v_ap, 1.0, None,
                                            OP.mult, OP.bypass)
                    if t == 0:
                        nc.vector.tensor_scalar(
                            aggs[0][:, sl], v_ap, 1.0, None,
                            OP.mult, OP.bypass)
                    else:
                        ns = min(t, TSEG - 1) + 1
                        for s in range(ns):
                            if s == t:
                                nc.gpsimd.scalar_tensor_tensor(
                                    aggs[s][:, sl], seg[:, sl],
                                    0.5 * s, vb_t[:, :F],
                                    OP.is_equal, OP.mult)
                            else:
                                mv_t = mvpool.tile([128, F], F16,
                                                   tag=f"mv{s}")
                                nc.gpsimd.scalar_tensor_tensor(
                                    mv_t[:, :F], seg[:, sl], 0.5 * s,
                                    vb_t[:, :F], OP.is_equal, OP.mult)
                                nc.vector.tensor_tensor(
                                    aggs[s][:, sl], aggs[s][:, sl],
                                    mv_t[:, :F], OP.add)
                    # seg += spike, after the masks read it (fp16 TT on DVE
                    # runs in 2x_1p; keeps Pool for the masks)
                    if t == 0:
                        nc.scalar.activation(seg[:, sl], spk_sl, AF.Copy)
                    else:
                        nc.vector.tensor_tensor(
                            seg[:, sl], seg[:, sl], spk_sl, OP.add)

                # p2(t, c) must issue before p1(t+1, c): lag strictly
                # less than chunks-per-step (and < vpool bufs)
                PKEEP = min(3, 2 * len(chunks) - 1)
                pending = []    # deferred phase-2 work

                def drain_pending(keep):
                    while len(pending) > keep:
                        args = pending.pop(0)
                        do_phase2(*args)

                for t in range(T):
                    bt = b * T + t
                    if t == 0:
                        issue_ev(0, bt)
                        issue_ev(1, bt)

                    for hf in range(2):
                        ss = sss[hf]
                        h0 = hf * HR
                        ss_r = ss[:].rearrange(
                            "p (hb r x) -> p hb r x", hb=4, r=HR, x=WP)
                        if len(chunks) > 2:
                            corder = ([chunks[1], chunks[0]]
                                      + chunks[2:])
                        elif len(chunks) == 2:
                            corder = [chunks[1], chunks[0]]
                        else:
                            corder = chunks
                        for ci, (c0, cr) in enumerate(corder):
                            F = cr * W
                            g0 = h0 + c0            # block-row of chunk start
                            sl = slice(g0 * W, g0 * W + F)
                            nsl = (cr + SR - 1) // SR

                            # ---- conv ----
                            # t==0: spikes are all zero; only the event+bias
                            # rows (96:109) contribute
                            kb = 96 if t == 0 else 0
                            cur_t = gcpool.tile([128, 2 * SR * W], F32,
                                                tag="cur")
                            gate_t = None
                            if t > 0:
                                gate_t = gcpool.tile([128, 2 * SR * W], F32,
                                                     tag="gate")
                            srr0 = min(SR, cr)
                            for hp_ in range(2):
                                ps = ppool.tile([128, nsl * 512], F32,
                                                tag=f"ps{hp_}")
                                for hbi in range(2):
                                    hb = 2 * hp_ + hbi
                                    tp = (kb, 64 * hbi)
                                    if kb == 0 and hbi == 0:
                                        tp = None
                                    pb = 64 * hbi
                                    nr, isl = 0, 0
                                    while nr < cr:
                                        srr = min(SR, cr - nr)
                                        out_ap = ps[pb:pb + 64,
                                                    isl * 512:
                                                    isl * 512 + srr * W]
                                        first = True
                                        for p_ in range(2):
                                            for kx in range(3):
                                                blk = 3 * p_ + kx
                                                mv_ap = ss_r[kb:KROWS, hb,
                                                             c0 + nr:
                                                             c0 + nr + srr,
                                                             kx:kx + W]
                                                nc.tensor.matmul(
                                                    out_ap,
                                                    ww_t[kb:KROWS,
                                                         64 * blk:
                                                         64 * (blk + 1)],
                                                    mv_ap,
                                                    start=first,
                                                    stop=(p_ == 1 and kx == 2),
                                                    tile_position=tp)
                                                first = False
                                        nr += srr
                                        isl += 1
                                # extraction: per-hb [32, nsl, srrW] ops
                                # (engine APs must start 32-aligned and
                                # span <=32 partitions from base 32/96)
                                R = nsl * 512
                                cv = cur_t[:].rearrange(
                                    "p (n x) -> p n x", x=srr0 * W)
                                gv = (gate_t[:].rearrange(
                                    "p (n x) -> p n x", x=srr0 * W)
                                    if t > 0 else None)
                                for hbi in range(2):
                                    hb = 2 * hp_ + hbi
                                    pb = 64 * hbi
                                    gsl = slice(32 * hb, 32 * (hb + 1))
                                    ps_c = mkap(
                                        ps[:], ps[:].offset + (pb + 32) * R,
                                        [(R, 32), (512, nsl),
                                         (1, srr0 * W)])
                                    nc.scalar.activation(
                                        cv[gsl, 0:nsl], ps_c, AF.Identity)
                                    if t > 0:
                                        ps_g = mkap(
                                            ps[:], ps[:].offset + pb * R,
                                            [(R, 32), (512, nsl),
                                             (1, srr0 * W)])
                                        nc.scalar.activation(
                                            gv[gsl, 0:nsl], ps_g,
                                            AF.Sigmoid)

                            # ---- v + spike (phase 1, latency-critical:
                            # next step's gathers wait on the spikes) ----
                            if t == 0:
                                v_t = None
                                v_ap = cur_t[:, :F]   # vmem==0 -> v = cur
                            else:
                                v_t = vpool.tile([128, F], F32, tag="v")
                                nc.vector.scalar_tensor_tensor(
                                    v_t[:, :F], gate_t[:, :F], 0.0,
                                    vmem[:, sl], OP.bypass, OP.mult)
                                nc.vector.scalar_tensor_tensor(
                                    v_t[:, :F], cur_t[:, :F], 0.0,
                                    v_t[:, :F], OP.bypass, OP.add)
                                v_ap = v_t[:, :F]
                            spk_sl = spk_v[:, g0 + 1:g0 + 1 + cr, 1:1 + W]
                            nc.vector.tensor_scalar(spk_sl, v_ap,
                                                    THRESH, 0.5,
                                                    OP.is_gt, OP.mult)
                            # t==0: v aliases the gc tile (bufs=2), so its
                            # phase-2 use cannot be deferred across chunks
                            if t == 0:
                                do_phase2(t, sl, F, cur_t[:, :F], spk_sl)
                            else:
                                pending.append((t, sl, F, v_t[:, :F],
                                                spk_sl))
                                drain_pending(PKEEP)

                        # ---- next-step gathers that only need this half
                        if t + 1 < T:
                            if hf == 0:
                                issue_gather(0, 1, 0, HR)
                                issue_gather(0, 0, 1, HR)
                                issue_gather(0, 2, 0, HR - 1)
                                issue_ev(0, bt + 1)
                            else:
                                issue_gather(0, 2, HR - 1, HR)
                                issue_top_edge()
                                issue_gather(1, 0, 0, HR)
                                issue_gather(1, 1, 0, HR)
                                issue_gather(1, 2, 0, HR - 1)
                                issue_bot_edge()
                                issue_ev(1, bt + 1)

                        if t == T - 1:
                            drain_pending(0)
                            issue_out(h0, h0 + HR)
    _split_matmul_waits(nc)
    return nc


def _split_matmul_waits(nc):
    """Walrus's LDW+MATMUL pair (and 2D DMA descriptors) have a single
    sync-wait slot; move extra waits onto same-engine no-ops inserted just
    before the instruction (safe: waits execute in order on the sequencer)."""
    nid = [0]
    for blk in nc.m.functions[0].blocks:
        out = []
        for inst in blk.instructions:
            si = inst.sync_info
            if (type(inst).__name__ != 'InstNoOp' and si is not None
                    and len(si.on_wait) > 1):
                keep = si.on_wait[-1:]
                for w in si.on_wait[:-1]:
                    nop = mybir.InstNoOp(name=f"NW-{nid[0]}", ins=[], outs=[])
                    nid[0] += 1
                    nop.engine = inst.engine
                    zupd = mybir.SyncUpdate(
                        sync_type='semaphore', id=nc.nop_sem_num,
                        ant_name='nopsem', update_mode='sem-inc',
                        update_value=1, update_reg=None)
                    nop.sync_info = mybir.SyncInfo(on_wait=[w],
                                                   on_update=[zupd])
                    out.append(nop)
                inst.sync_info = mybir.SyncInfo(on_wait=keep,
                                                on_update=si.on_update)
            out.append(inst)
        blk.instructions = out


def host_prep(events, w_in, b_in, w_gate, b_gate, conv_np=np.float32,
              ncores=NCORES, mode=None):
    """Build per-core input maps. events: [B,T,CIN,H,W] full."""
    Bf, Tf, Cf, Hf, Wf = events.shape
    HP, WP = Hf + 2, Wf + 2
    # time reversal + zero pad
    evr = events[:, ::-1].astype(np.float32)
    evp = np.zeros((Bf, Tf, Cf, HP, WP), np.float32)
    evp[..., 1:1 + Hf, 1:1 + Wf] = evr
    hi = evp.astype(np.float16)
    lo = (evp - hi.astype(np.float32)).astype(np.float16)
    # ky-shifted stacked planes: evs[:,:,3*ch+ky, r] = src_ch[r+ky]
    evs = np.zeros((Bf, Tf, 12, HP, WP), np.float16)
    for ch in range(4):
        src = hi[:, :, ch] if ch < 2 else lo[:, :, ch - 2]
        for ky in range(3):
            evs[:, :, 3 * ch + ky, 0:HP - ky] = src[:, :, ky:HP]

    # stationary weights [128, 384]: col block bk = 3*pass + kx; row 108 is
    # the bias row (pairs with the constant-1.0 moving row)
    w2 = 2.0 * np.asarray(w_gate, np.float32)          # [64, 32, 3, 3]
    w2hi = w2.astype(np.float16)
    w2lo = (w2 - w2hi.astype(np.float32)).astype(np.float16)
    wi = np.asarray(w_in, np.float32)                  # [64, 2, 3, 3]
    wih = wi.astype(np.float16)
    wil = (wi - wih.astype(np.float32)).astype(np.float16)
    bsum = (np.asarray(b_gate, np.float32)
            + np.asarray(b_in, np.float32))            # [64]
    bhi = bsum.astype(np.float16)
    blo = (bsum - bhi.astype(np.float32)).astype(np.float16)
    ww = np.zeros((128, 384), np.float16)
    for p_ in range(2):
        wg_src = w2hi if p_ == 0 else w2lo
        wi_src = wih if p_ == 0 else wil
        for kx in range(3):
            c0 = 64 * (3 * p_ + kx)
            for ky in range(3):
                for c in range(COUT):
                    ww[32 * ky + c, c0:c0 + 64] = wg_src[:, c, ky, kx]
                for ch in range(4):
                    cin = ch % 2
                    ww[96 + 3 * ch + ky, c0:c0 + 64] = wi_src[:, cin, ky, kx]
            # bias row contributes once per pass (kx==1 block)
            if kx == 1:
                ww[108, c0:c0 + 64] = bhi if p_ == 0 else blo

    bl = Bf // ncores
    in_maps = []
    for i in range(ncores):
        ev_i = evs[i * bl:(i + 1) * bl].reshape(bl * Tf, 12, HP * WP)
        in_maps.append({"ev": np.ascontiguousarray(ev_i), "ww": ww})
    return in_maps


_cache = {}
last_run_info = {}


def kernel(events, w_in, b_in, w_gate, b_gate, trace=False):
    from concourse import bass_utils
    key = ("v4",)
    if key not in _cache:
        _cache[key] = build_nc()
    nc = _cache[key]
    in_maps = host_prep(np.asarray(events), np.asarray(w_in),
                        np.asarray(b_in), np.asarray(w_gate),
                        np.asarray(b_gate))
    t0 = time.time()
    res = bass_utils.run_bass_kernel_spmd(
        nc, in_maps, core_ids=list(range(NCORES)), trace=trace)
    wall = time.time() - t0
    last_run_info.update(exec_time_ns=res.exec_time_ns, wall_s=wall,
                         profile_json=getattr(res, "profile_json", None))
    outs = [np.asarray(res.results[i]["out"]).astype(np.float32)
            for i in range(NCORES)]
    return np.concatenate(outs, axis=1)
